# revision 1
# baseline (speedup 1.0000x reference)
"""DIEN (GRU + AUGRU scan) Trainium2 Bass kernel — bf16 pipelined version.

Strategy
--------
Data-parallel over batch: B=256 split 8 ways (32 per core); weights replicated;
the T=200 scan runs locally per core.

Algebraic fusion (host, exact): per-step attention is over a length-1 sequence,
so softmax==1 and attn == v_proj(g). Folding v_proj into the AUGRU input
weights removes that GEMM entirely.

Per step, 4 GEMM groups of [32,512]@[512,1536] run on the PE in bf16
(1 cycle/row vs 4 for fp32 -> 4x). Batch (32) is the stationary free dim with
4x PE column tiling; the 4 column FIFOs run concurrently, so a group costs
4 k-tiles x 384 rows = 1536 PE cycles (~640 ns). PE-roofline/step ~2.56 us.

Software pipeline (period P, steady state), PE order:
  GRUhid(i+1) | AUGRUin(i) | xproj(i+2) | AUGRUhid(i)
Hidden groups emit all rz matmuls before n matmuls so the cells' sigmoids
start early. Cell element-wise chains are spread over ACT/DVE/POOL so both
cells' latencies hide under the other groups' PE time.

Biases are pre-accumulated into PSUM off the critical path: xproj/AUGRUin
start the psum bank, then an engine adds the (host-prearranged) bias tile
in-place, then the hidden-side rz matmuls accumulate on top (start=False).
The hidden n parts go to separate small psum banks (r must gate them before
tanh); their bias joins via the u = ghn + bhh_n op that feeds r*u.

Layouts (per core, batch b 0..31, hidden h = 128*c + 32*m + jr):
  row layout  : tile[32*c + b, 32*m + jr]  (states, psum outputs)
  stationary  : tileT[32*c + jr, 32*m + b] = one DVE 32x32 block transpose.
  x is pre-transposed to stationary layout on host and DMAed in chunks.
"""

import os
import sys

import numpy as np

for _p in ("/opt/trn_rl_repo", "/root/.axon_site/_ro/trn_rl_repo"):
    if os.path.isdir(_p) and _p not in sys.path:
        sys.path.append(_p)

import ml_dtypes

BF16NP = ml_dtypes.bfloat16

B, T, H = 256, 200, 512
N_CORES = 8
BL = B // N_CORES  # 32
CHUNK = 8          # timesteps per x DMA chunk
NCHUNK = T // CHUNK

_CACHE = {}


# ---------------------------------------------------------------------------
# Host-side weight preparation (pure numpy, exact rearrangements)
# ---------------------------------------------------------------------------

def _arrange_w(W, xside):
    """[3H, H] (out, in) -> [128, 4, 3H] K-tile-arranged weight blocks.

    Partition p = 32*c_in + jr holds input dim h_in = 128*c_in + 32*m + jr for
    K-tile m. Free f = c_out*384 + slot*128 + j. PSUM layout per c_out block is
    [gin | r | z | hn] (4x128); the x-side writes slots (gin,r,z) = gates
    (n,r,z), the h-side writes slots (r,z,hn) = gates (r,z,n). Both are
    contiguous 384-wide windows, so each (k,c) is a single matmul.
    """
    A = W.T.reshape(4, 4, 32, 3 * H)                # [c_in, m, jr, out]
    A = A.transpose(1, 0, 2, 3).reshape(4, 128, 3 * H)
    A = A.reshape(4, 128, 3, 4, 128)                # [m, p, gate(r,z,n), c, j]
    order = (2, 0, 1) if xside else (0, 1, 2)
    A = A[:, :, order, :, :]
    A = A.transpose(0, 1, 3, 2, 4).reshape(4, 128, 3 * H)
    A = A.transpose(1, 0, 2)                        # [p, m, out]
    return np.ascontiguousarray(A)


def _bias_rz(bv):
    """[3H] -> [128, 256] broadcast tile for the r,z gate psum region."""
    v = bv[:1024].reshape(2, 4, 128).transpose(1, 0, 2).reshape(4, 256)
    return np.repeat(v, 32, axis=0)


def _bias_n(bv):
    """[3H] -> [128, 128] broadcast tile for an n gate psum region."""
    v = bv[1024:].reshape(4, 128)
    return np.repeat(v, 32, axis=0)


def _bias_full(bih, bhh):
    """[128, 512] PSUM prewrite tile: [gin_bias | rz_bias | hn_bias]."""
    return np.concatenate(
        [_bias_n(bih), _bias_rz(bih + bhh), _bias_n(bhh)], axis=1
    )


def _arrange_x(x):
    """[BL, nt, H] -> [nt, 128, 128] stationary-layout bf16 (p=32c+jr, f=32m+b)."""
    nt = x.shape[1]
    xt = x.reshape(BL, nt, 4, 4, 32).transpose(1, 2, 4, 3, 0).reshape(nt, 128, 128)
    return np.ascontiguousarray(xt.astype(BF16NP))


# ---------------------------------------------------------------------------
# Bass program
# ---------------------------------------------------------------------------

def _build_program(n_steps=T, repeat=1, xwrap=None):
    """repeat>1 wraps the scan in a hardware loop; xwrap=N makes step t read
    x[t % N] so long timing variants reuse the same x buffer. Both are timing
    tools only (numerics are only meaningful for the default arguments)."""
    import concourse.bacc as bacc
    import concourse.tile as tile
    from concourse import mybir
    import concourse.bass as bass_mod
    from contextlib import ExitStack, nullcontext

    F32 = mybir.dt.float32
    BF16 = mybir.dt.bfloat16
    Sigmoid = mybir.ActivationFunctionType.Sigmoid
    Tanh = mybir.ActivationFunctionType.Tanh
    MULT = mybir.AluOpType.mult
    ADD = mybir.AluOpType.add

    xsteps = xwrap if xwrap is not None else n_steps
    nchunk = (xsteps + CHUNK - 1) // CHUNK
    nc = bacc.Bacc("TRN2", target_bir_lowering=False, debug=False)

    xt_dram = nc.declare_dram_parameter("xt", [xsteps, 128, 128], BF16, isOutput=False)
    w_dram = {
        name: nc.declare_dram_parameter(name, [128, 4, 3 * H], BF16, isOutput=False)
        for name in ("wgi", "wgh", "wai", "wah")
    }
    b_dram = {
        "bias_g": nc.declare_dram_parameter("bias_g", [128, 512], F32, isOutput=False),
        "bias_a": nc.declare_dram_parameter("bias_a", [128, 512], F32, isOutput=False),
    }
    out = nc.declare_dram_parameter("out", [BL, H], F32, isOutput=True)

    with tile.TileContext(nc) as tc, ExitStack() as ctx:
        wpool = ctx.enter_context(tc.tile_pool(name="weights", bufs=1))
        st_pool = ctx.enter_context(tc.tile_pool(name="states", bufs=3))
        tmp_pool = ctx.enter_context(tc.tile_pool(name="tmps", bufs=3))
        ps_gi = ctx.enter_context(tc.tile_pool(name="psgi", bufs=3, space="PSUM"))
        ps_ai = ctx.enter_context(tc.tile_pool(name="psai", bufs=3, space="PSUM"))

        # --- constants: weights + biases + x chunks ---
        # DMA order matters for lead-in: step 0 needs wgi, biasx_g, xt chunk 0
        # first; everything else streams in behind the compute front.
        wsb, bsb, xt_sb = {}, {}, []

        def dma_w(name):
            t = wpool.tile([128, 4 * 3 * H], BF16, tag=name, name=name)
            nc.sync.dma_start(out=t, in_=w_dram[name][:].rearrange("p m f -> p (m f)"))
            wsb[name] = t

        def dma_b(name):
            drm = b_dram[name]
            t = wpool.tile([128, drm.shape[1]], F32, tag=name, name=name)
            nc.sync.dma_start(out=t, in_=drm[:])
            bsb[name] = t

        def dma_x(ch):
            t0 = ch * CHUNK
            t1 = min(t0 + CHUNK, xsteps)
            t = wpool.tile([128, (t1 - t0) * 128], BF16, tag=f"xt{ch}", name=f"xt{ch}")
            src = bass_mod.AP(
                tensor=xt_dram[:].tensor,
                offset=t0 * 128 * 128,
                ap=[[128, 128], [128 * 128, t1 - t0], [1, 128]],
            )
            nc.sync.dma_start(out=t, in_=src)
            xt_sb.append(t)

        dma_w("wgi")
        dma_b("bias_g")
        dma_x(0)
        dma_w("wgh")
        dma_w("wai")
        dma_b("bias_a")
        dma_w("wah")
        for ch in range(1, nchunk):
            dma_x(ch)

        def x_lhsT(t_, k):
            ch, off = divmod(t_ % xsteps, CHUNK)
            return xt_sb[ch][:, off * 128 + 32 * k: off * 128 + 32 * k + 32]

        def mm_x(psum, lhsT_fn, w):
            """x-side group: accumulates [0:384] = (gin|r|z) onto the
            engine-prewritten bias; all start=False."""
            for k in range(4):
                lhsT = lhsT_fn(k)
                for c in range(4):
                    base = k * 1536 + 384 * c
                    nc.tensor.matmul(
                        out=psum[32 * c:32 * c + 32, 0:384],
                        lhsT=lhsT,
                        rhs=w[:, base:base + 384],
                        start=False, stop=False,
                        skip_group_check=True, tile_position=(0, 32 * c),
                    )

        def mm_h_split(psum, statT, w):
            """Hidden-side group with all rz matmuls before the hn matmuls."""
            for k in range(4):
                lhsT = statT[:, 32 * k:32 * k + 32]
                for c in range(4):
                    base = k * 1536 + 384 * c
                    nc.tensor.matmul(
                        out=psum[32 * c:32 * c + 32, 128:384],
                        lhsT=lhsT,
                        rhs=w[:, base:base + 256],
                        start=False, stop=(k == 3),
                        skip_group_check=True, tile_position=(0, 32 * c),
                    )
            for k in range(4):
                lhsT = statT[:, 32 * k:32 * k + 32]
                for c in range(4):
                    base = k * 1536 + 384 * c
                    nc.tensor.matmul(
                        out=psum[32 * c:32 * c + 32, 384:512],
                        lhsT=lhsT,
                        rhs=w[:, base + 256:base + 384],
                        start=False, stop=(k == 3),
                        skip_group_check=True, tile_position=(0, 32 * c),
                    )

        def mm_h(psum, statT, w):
            """Hidden-side group: one matmul per (k,c) writing [128:512] =
            (r|z|hn) — rz accumulates onto the x side, hn onto the
            prewritten bhh_n bias."""
            for k in range(4):
                lhsT = statT[:, 32 * k:32 * k + 32]
                for c in range(4):
                    base = k * 1536 + 384 * c
                    nc.tensor.matmul(
                        out=psum[32 * c:32 * c + 32, 128:512],
                        lhsT=lhsT,
                        rhs=w[:, base:base + 384],
                        start=False, stop=(k == 3),
                        skip_group_check=True, tile_position=(0, 32 * c),
                    )

        # --- element-wise helpers ---
        # Engine rules: GPSIMD (Pool) may NOT touch PSUM. ACT: sigmoid/tanh +
        # PSUM->SBUF copies + the gi prewrite; DVE: v (reads hn psum), the
        # a-cell tail, transposes, the ai prewrite; Pool: SBUF-only math.
        # PSUM bank layout [0:128]=gin(x n), [128:384]=r|z, [384:512]=hn.

        def sig_of(p, tag):
            rz = tmp_pool.tile([128, 256], F32, tag=tag + "rz", name="rz")
            nc.scalar.activation(rz, p[:, 128:384], Sigmoid)
            return rz

        def ugin_of(p, tag):
            u = tmp_pool.tile([128, 128], F32, tag=tag + "ug", name="ug")
            nc.scalar.copy(u, p[:, 0:128])
            return u

        def v_of(rz, p, tag):
            v = tmp_pool.tile([128, 128], F32, tag=tag + "v", name="v")
            nc.vector.tensor_mul(v, rz[:, 0:128], p[:, 384:512])
            return v

        def t3_of(v, ugin, tag):
            t3 = tmp_pool.tile([128, 128], F32, tag=tag + "t3", name="t3")
            nc.gpsimd.tensor_add(t3, v, ugin)
            return t3

        def tanh_of(t3, tag):
            n = tmp_pool.tile([128, 128], F32, tag=tag + "n", name="n")
            nc.scalar.activation(n, t3, Tanh)
            return n

        def omz_of(rz, tag):
            omz = tmp_pool.tile([128, 128], F32, tag=tag + "omz", name="omz")
            nc.gpsimd.tensor_scalar(omz, rz[:, 128:256], -1.0, 1.0, MULT, ADD)
            return omz

        def zh_of(rz, row_prev, tag):
            zh = tmp_pool.tile([128, 128], F32, tag=tag + "zh", name="zh")
            nc.gpsimd.tensor_mul(zh, rz[:, 128:256], row_prev)
            return zh

        def tail_of(omz, zh, n, tag, eng):
            row = st_pool.tile([128, 128], BF16, tag=tag + "row", name="row")
            if zh is None:
                eng.tensor_mul(row, omz, n)
            else:
                m1 = tmp_pool.tile([128, 128], F32, tag=tag + "m1", name="m1")
                eng.tensor_mul(m1, omz, n)
                eng.tensor_add(row, m1, zh)
            rowT = st_pool.tile([128, 128], BF16, tag=tag + "rowT", name="rowT")
            nc.vector.transpose(rowT, row)
            return row, rowT

        def prewrite_gi():
            """New gi bank pre-filled with [gin|rz|hn] biases, on ACT."""
            p = ps_gi.tile([128, 512], F32, tag="gi", name="gi")
            nc.scalar.copy(p, bsb["bias_g"])
            return p

        def prewrite_ai_a():
            """New ai bank; first bias half on DVE (second half emitted later
            so trT_g can slot between the two copies)."""
            p = ps_ai.tile([128, 512], F32, tag="ai", name="ai")
            nc.vector.tensor_copy(p[:, 0:256], bsb["bias_a"][:, 0:256])
            return p

        def prewrite_ai_b(p):
            nc.vector.tensor_copy(p[:, 256:512], bsb["bias_a"][:, 256:512])

        # --- software-pipelined scan ---
        pgi = {}
        pai_cur = None
        pai_next = None
        pai_prev = None
        ugin_a_prev = None
        first_a = True      # cell_a(0) has zero initial hidden state
        g_row = {}
        gT = {}
        a_row = {}
        aT = {}

        # prologue: prewrite gi(0..2), xproj(0..1), prewrite ai(0), cell_g(0)
        for s in range(min(3, n_steps)):
            pgi[s] = prewrite_gi()
        mm_x(pgi[0], lambda k: x_lhsT(0, k), wsb["wgi"])
        if n_steps > 1:
            mm_x(pgi[1], lambda k: x_lhsT(1, k), wsb["wgi"])
        pai_cur = prewrite_ai_a()
        prewrite_ai_b(pai_cur)

        # cell_g(0): zero hidden -> no GRUhid matmuls; psum[384:512] = bhh_n.
        rz0 = sig_of(pgi[0], "g")
        ugin0 = ugin_of(pgi[0], "g")
        v0 = v_of(rz0, pgi[0], "g")
        t30 = t3_of(v0, ugin0, "g")
        n0 = tanh_of(t30, "g")
        omz0 = omz_of(rz0, "g")
        g_row[0], gT[0] = tail_of(omz0, None, n0, "g", nc.gpsimd)
        del pgi[0]

        def emit_scan():
            nonlocal pai_cur, pai_next, pai_prev, ugin_a_prev, first_a
            for i in range(n_steps + 1):
                # -- PE: GRUhid(i+1) (one 16-mm group onto pgi(i+1)[128:512])
                if i + 1 < n_steps:
                    mm_h(pgi[i + 1], gT[i], wsb["wgh"])

                # -- ACT: sig_a(i-1); DVE: v_a(i-1)
                if i >= 1:
                    rz_a = sig_of(pai_prev, "a")
                    v_a = v_of(rz_a, pai_prev, "a")

                # -- POOL: omz_a, zh_a, t3_a; ACT: ugin_g(i+1), sig_g(i+1)
                if i >= 1:
                    omz_a = omz_of(rz_a, "a")
                    zh_a = None if first_a else zh_of(rz_a, a_row[i - 2], "a")
                    t3_a = t3_of(v_a, ugin_a_prev, "a")
                if i + 1 < n_steps:
                    ugin_g = ugin_of(pgi[i + 1], "g")
                    rz_g = sig_of(pgi[i + 1], "g")

                # -- PE: AUGRUin(i)
                if i < n_steps:
                    mm_x(pai_cur, lambda k, i=i: (gT[i][:, 32 * k:32 * k + 32]),
                         wsb["wai"])

                # -- DVE: v_g(i+1); ACT: tanh_a(i-1)
                if i + 1 < n_steps:
                    v_g = v_of(rz_g, pgi[i + 1], "g")
                if i >= 1:
                    n_a = tanh_of(t3_a, "a")

                # -- POOL: t3_g, omz_g, zh_g
                if i + 1 < n_steps:
                    t3_g = t3_of(v_g, ugin_g, "g")
                    omz_g = omz_of(rz_g, "g")
                    zh_g = zh_of(rz_g, g_row[i], "g")

                # -- DVE: m1_a, row_a, trT_a; ACT: ugin_a(i)
                if i >= 1:
                    a_row[i - 1], aT[i - 1] = tail_of(omz_a, zh_a, n_a, "a",
                                                      nc.vector)
                    first_a = False
                    a_row.pop(i - 3, None)
                    aT.pop(i - 2, None)
                if i < n_steps:
                    ugin_a_cur = ugin_of(pai_cur, "a")

                # -- PE: xproj(i+2); ACT: prewrite_gi(i+3)
                if i + 2 < n_steps:
                    mm_x(pgi[i + 2], lambda k, s=i + 2: x_lhsT(s, k), wsb["wgi"])
                if i + 3 < n_steps:
                    pgi[i + 3] = prewrite_gi()

                # -- ACT: tanh_g; DVE: prewrite_ai(i+1) (first half)
                if i + 1 < n_steps:
                    n_g = tanh_of(t3_g, "g")
                    pai_next = prewrite_ai_a()

                # -- PE: AUGRUhid(i) (skipped for i==0), rz before hn so
                # cell_a(i) can start its sigmoid ~2 k-passes earlier
                if 1 <= i < n_steps:
                    mm_h_split(pai_cur, aT[i - 1], wsb["wah"])

                # -- POOL: m1_g, row_g; DVE: trT_g, then 2nd ai bias half
                if i + 1 < n_steps:
                    g_row[i + 1], gT[i + 1] = tail_of(omz_g, zh_g, n_g, "g",
                                                      nc.gpsimd)
                    prewrite_ai_b(pai_next)
                    del pgi[i + 1]
                    g_row.pop(i - 1, None)
                    gT.pop(i, None)

                # -- rotate A-cell pipeline state
                if i < n_steps:
                    pai_prev = pai_cur
                    ugin_a_prev = ugin_a_cur
                    pai_cur = pai_next

        loop_cm = tc.For_i(0, repeat, 1) if repeat > 1 else nullcontext()
        with loop_cm:
            emit_scan()

        # epilogue: final AUGRU state -> fp32 -> DRAM
        final = a_row[n_steps - 1]
        out_row = tmp_pool.tile([128, 128], F32, tag="outrow")
        nc.scalar.copy(out_row, final)
        out_ap = bass_mod.AP(
            tensor=out[:].tensor,
            offset=0,
            ap=[[128, 4], [H, BL], [1, 128]],
        )
        nc.sync.dma_start(out=out_ap, in_=out_row)

    nc.compile()
    return nc


def _get_program(n_steps=T):
    key = ("prog", n_steps)
    if key not in _CACHE:
        _CACHE[key] = _build_program(n_steps)
    return _CACHE[key]


# ---------------------------------------------------------------------------
# Entry point
# ---------------------------------------------------------------------------

def _make_consts(inputs):
    augru_Wih = np.asarray(inputs["augru_Wih"], np.float64)
    A1 = augru_Wih[:, :H]
    A2 = augru_Wih[:, H:]
    w_fused = A1 + A2 @ np.asarray(inputs["v_W"], np.float64)
    b_ai = np.asarray(inputs["augru_bih"], np.float64) + A2 @ np.asarray(
        inputs["v_b"], np.float64
    )
    b_ah = np.asarray(inputs["augru_bhh"], np.float64)
    gru_bih = np.asarray(inputs["gru_bih"], np.float64)
    gru_bhh = np.asarray(inputs["gru_bhh"], np.float64)

    def f32(x):
        return np.ascontiguousarray(x, dtype=np.float32)

    def bf(x):
        return np.ascontiguousarray(x.astype(np.float32)).astype(BF16NP)

    return {
        "wgi": bf(_arrange_w(np.asarray(inputs["gru_Wih"], np.float64), True)),
        "wgh": bf(_arrange_w(np.asarray(inputs["gru_Whh"], np.float64), False)),
        "wai": bf(_arrange_w(w_fused, True)),
        "wah": bf(_arrange_w(np.asarray(inputs["augru_Whh"], np.float64), False)),
        "bias_g": f32(_bias_full(gru_bih, gru_bhh)),
        "bias_a": f32(_bias_full(b_ai, b_ah)),
    }


def _make_in_maps(inputs):
    seq_emb = np.asarray(inputs["seq_emb"], np.float32)
    consts = _make_consts(inputs)
    return [
        {"xt": _arrange_x(seq_emb[c * BL:(c + 1) * BL]), **consts}
        for c in range(N_CORES)
    ]


def _prep_and_run(trace=False, **inputs):
    from concourse.bass_utils import run_bass_kernel_spmd

    in_maps = _make_in_maps(inputs)
    nc = _get_program()
    res = run_bass_kernel_spmd(nc, in_maps, list(range(N_CORES)), trace=trace)
    out = np.concatenate([res.results[c]["out"] for c in range(N_CORES)], axis=0)
    return out.astype(np.float32), res


def kernel(**inputs):
    out, _ = _prep_and_run(**inputs)
    return out


def kernel_traced(**inputs):
    """Like kernel() but profiles the run; returns (output, BassKernelResults)."""
    return _prep_and_run(**inputs, trace=True)



# revision 2
# speedup vs baseline: 7.1873x; 7.1873x over previous
"""DIEN (GRU + AUGRU scan) Trainium2 Bass kernel — v2 split-bank pipeline.

Strategy
--------
Data-parallel over batch: B=256 split 8 ways (32 per core); weights replicated;
the T=200 scan runs locally per core. Attention v_proj folded into the AUGRU
input weights (exact; per-step softmax over length-1 sequence == 1).

v2 changes vs the prewrite/single-bank version:
- Each cell's psum is split across TWO banks: A = [gin | r | z] (x-side +
  hidden rz accumulate there), B = [hn] alone. PSUM bank R/W collisions are
  fatal and Tile orders conservatively per bank, so with hn in its own bank
  the sigmoid over [r|z] can run while the hn matmuls still stream.
- gin is read straight out of PSUM by the t3 add on DVE (no ACT ugin copy).
- A-bank biases are engine-prewritten (ACT for the GRU, DVE for the AUGRU)
  from host-broadcast [128,384] tiles; B-bank biases arrive via a K=1
  ones-row matmul that also opens the bank (start=True).
- Post-tanh tail runs in bf16 (DVE 2x mode for the g cell, POOL for the a
  cell); sigmoid outputs fp32 (r feeds the fp32 v mul against PSUM).

Per-step engine budget (HW model, ns): PE ~2850 (4 GEMM groups, 4x column
tiling, + 8 K=1 bias mms), ACT ~1810, DVE ~2420, POOL ~1940. The serial
g-recurrence (ghid rz -> sig -> v -> t3 -> tanh -> m1 -> row -> transpose)
fits inside the PE period with ~250ns slack.

Layouts (per core, batch b 0..31, hidden h = 128*c + 32*m + jr):
  row layout  : tile[32*c + b, 32*m + jr]  (states, psum outputs)
  stationary  : tileT[32*c + jr, 32*m + b] = one DVE 32x32 block transpose.
  x is pre-transposed to stationary layout on host and DMAed in chunks.
"""

import os
import sys

import numpy as np

for _p in ("/opt/trn_rl_repo", "/root/.axon_site/_ro/trn_rl_repo"):
    if os.path.isdir(_p) and _p not in sys.path:
        sys.path.append(_p)

import ml_dtypes

BF16NP = ml_dtypes.bfloat16

B, T, H = 256, 200, 512
N_CORES = 8
BL = B // N_CORES  # 32
CHUNK = 8          # timesteps per x DMA chunk
NCHUNK = T // CHUNK

_CACHE = {}


# ---------------------------------------------------------------------------
# Host-side weight preparation (pure numpy, exact rearrangements)
# ---------------------------------------------------------------------------

def _arrange_w(W, xside):
    """[3H, H] (out, in) -> [128, 4, 3H] K-tile-arranged weight blocks.

    Partition p = 32*c_in + jr holds input dim h_in = 128*c_in + 32*m + jr for
    K-tile m. Free f = c_out*384 + slot*128 + j. PSUM layout: bank A is
    [gin | r | z] (slots 0,1,2 for the x side = gates n,r,z), bank B is [hn].
    The h side keeps slot order (r,z,n): its rz window is cols [128:384) of
    each 384 block, its n window is cols [256:384) -> bank B.
    """
    A = W.T.reshape(4, 4, 32, 3 * H)                # [c_in, m, jr, out]
    A = A.transpose(1, 0, 2, 3).reshape(4, 128, 3 * H)
    A = A.reshape(4, 128, 3, 4, 128)                # [m, p, gate(r,z,n), c, j]
    order = (2, 0, 1) if xside else (0, 1, 2)
    A = A[:, :, order, :, :]
    A = A.transpose(0, 1, 3, 2, 4).reshape(4, 128, 3 * H)
    A = A.transpose(1, 0, 2)                        # [p, m, out]
    return np.ascontiguousarray(A)


def _bias_A(bih, bhh):
    """[3H],[3H] -> [128, 384] f32 broadcast tile for bank A = [gin | r | z].

    gin gets bih_n; r and z get bih+bhh (both sides' matmuls accumulate in A).
    """
    rz = (bih + bhh)
    r = rz[0:512].reshape(4, 128)
    z = rz[512:1024].reshape(4, 128)
    gin = bih[1024:1536].reshape(4, 128)
    v = np.concatenate([gin, r, z], axis=1)         # [4, 384] per c block
    return np.repeat(v, 32, axis=0)                 # [128, 384]


def _bias_B_row(bhh):
    """[3H] -> [1, 512] row for the K=1 hn-bias matmul (bhh_n, c-major)."""
    return np.ascontiguousarray(bhh[1024:1536].reshape(1, 512))


def _arrange_x(x):
    """[BL, nt, H] -> [nt, 128, 128] stationary-layout bf16 (p=32c+jr, f=32m+b)."""
    nt = x.shape[1]
    xt = x.reshape(BL, nt, 4, 4, 32).transpose(1, 2, 4, 3, 0).reshape(nt, 128, 128)
    return np.ascontiguousarray(xt.astype(BF16NP))


# ---------------------------------------------------------------------------
# Bass program
# ---------------------------------------------------------------------------

def _build_program(n_steps=T, repeat=1, xwrap=None):
    """repeat>1 wraps the scan in a hardware loop; xwrap=N makes step t read
    x[t % N] so long timing variants reuse the same x buffer. Both are timing
    tools only (numerics are only meaningful for the default arguments)."""
    import concourse.bacc as bacc
    import concourse.tile as tile
    from concourse import mybir
    import concourse.bass as bass_mod
    from contextlib import ExitStack, nullcontext

    F32 = mybir.dt.float32
    BF16 = mybir.dt.bfloat16
    Sigmoid = mybir.ActivationFunctionType.Sigmoid
    Tanh = mybir.ActivationFunctionType.Tanh
    MULT = mybir.AluOpType.mult
    ADD = mybir.AluOpType.add

    xsteps = xwrap if xwrap is not None else n_steps
    nchunk = (xsteps + CHUNK - 1) // CHUNK
    nc = bacc.Bacc("TRN2", target_bir_lowering=False, debug=False)

    xt_dram = nc.declare_dram_parameter("xt", [xsteps, 128, 128], BF16, isOutput=False)
    w_dram = {
        name: nc.declare_dram_parameter(name, [128, 4, 3 * H], BF16, isOutput=False)
        for name in ("wgi", "wgh", "wai", "wah")
    }
    bA_dram = {
        "biasA_g": nc.declare_dram_parameter("biasA_g", [128, 384], F32, isOutput=False),
        "biasA_a": nc.declare_dram_parameter("biasA_a", [128, 384], F32, isOutput=False),
    }
    # small consts, single partition: [ones(32) | biasB_g(512) | biasB_a(512)]
    small_dram = nc.declare_dram_parameter("small", [1, 1056], BF16, isOutput=False)
    out = nc.declare_dram_parameter("out", [BL, H], F32, isOutput=True)

    with tile.TileContext(nc) as tc, ExitStack() as ctx:
        wpool = ctx.enter_context(tc.tile_pool(name="weights", bufs=1))
        st_pool = ctx.enter_context(tc.tile_pool(name="states", bufs=4))
        tmp_pool = ctx.enter_context(tc.tile_pool(name="tmps", bufs=3))
        ps_giA = ctx.enter_context(tc.tile_pool(name="psgiA", bufs=2, space="PSUM"))
        ps_giB = ctx.enter_context(tc.tile_pool(name="psgiB", bufs=2, space="PSUM"))
        ps_aiA = ctx.enter_context(tc.tile_pool(name="psaiA", bufs=2, space="PSUM"))
        ps_aiB = ctx.enter_context(tc.tile_pool(name="psaiB", bufs=2, space="PSUM"))

        # --- constants: weights + biases + x chunks ---
        wsb, bsb, xt_sb = {}, {}, []

        def dma_w(name):
            t = wpool.tile([128, 4 * 3 * H], BF16, tag=name, name=name)
            nc.sync.dma_start(out=t, in_=w_dram[name][:].rearrange("p m f -> p (m f)"))
            wsb[name] = t

        def dma_bA(name):
            drm = bA_dram[name]
            t = wpool.tile([128, 384], F32, tag=name, name=name)
            nc.sync.dma_start(out=t, in_=drm[:])
            bsb[name] = t

        def dma_x(ch):
            t0 = ch * CHUNK
            t1 = min(t0 + CHUNK, xsteps)
            t = wpool.tile([128, (t1 - t0) * 128], BF16, tag=f"xt{ch}", name=f"xt{ch}")
            src = bass_mod.AP(
                tensor=xt_dram[:].tensor,
                offset=t0 * 128 * 128,
                ap=[[128, 128], [128 * 128, t1 - t0], [1, 128]],
            )
            nc.sync.dma_start(out=t, in_=src)
            xt_sb.append(t)

        small_sb = wpool.tile([1, 1056], BF16, tag="small", name="small")
        nc.sync.dma_start(out=small_sb, in_=small_dram[:])
        ones32 = small_sb[0:1, 0:32]
        brow = {"g": small_sb[0:1, 32:544], "a": small_sb[0:1, 544:1056]}

        dma_w("wgi")
        dma_bA("biasA_g")
        dma_x(0)
        dma_w("wgh")
        dma_w("wai")
        dma_bA("biasA_a")
        dma_w("wah")
        for ch in range(1, nchunk):
            dma_x(ch)

        def x_lhsT(t_, k):
            ch, off = divmod(t_ % xsteps, CHUNK)
            return xt_sb[ch][:, off * 128 + 32 * k: off * 128 + 32 * k + 32]

        def mm_x(psumA, lhsT_fn, w):
            """x-side group: accumulates A[0:384] = (gin|r|z) onto the
            engine-prewritten bias; all start=False."""
            for k in range(4):
                lhsT = lhsT_fn(k)
                for c in range(4):
                    base = k * 1536 + 384 * c
                    nc.tensor.matmul(
                        out=psumA[32 * c:32 * c + 32, 0:384],
                        lhsT=lhsT,
                        rhs=w[:, base:base + 384],
                        start=False, stop=False,
                        skip_group_check=True, tile_position=(0, 32 * c),
                    )

        def mm_h_rz(psumA, statT, w):
            """Hidden-side rz accumulate onto bank A [128:384]."""
            for k in range(4):
                lhsT = statT[:, 32 * k:32 * k + 32]
                for c in range(4):
                    base = k * 1536 + 384 * c
                    nc.tensor.matmul(
                        out=psumA[32 * c:32 * c + 32, 128:384],
                        lhsT=lhsT,
                        rhs=w[:, base:base + 256],
                        start=False, stop=(k == 3),
                        skip_group_check=True, tile_position=(0, 32 * c),
                    )

        def mm_h_hn(psumB, statT, w):
            """Hidden-side hn accumulate onto bank B [0:128] (over bias)."""
            for k in range(4):
                lhsT = statT[:, 32 * k:32 * k + 32]
                for c in range(4):
                    base = k * 1536 + 384 * c
                    nc.tensor.matmul(
                        out=psumB[32 * c:32 * c + 32, 0:128],
                        lhsT=lhsT,
                        rhs=w[:, base + 256:base + 384],
                        start=False, stop=(k == 3),
                        skip_group_check=True, tile_position=(0, 32 * c),
                    )

        def mm_biasB(psumB, cell):
            """K=1 ones-row matmul: opens bank B with the bhh_n bias."""
            for c in range(4):
                nc.tensor.matmul(
                    out=psumB[32 * c:32 * c + 32, 0:128],
                    lhsT=ones32,
                    rhs=brow[cell][0:1, 128 * c:128 * c + 128],
                    start=True, stop=False,
                    skip_group_check=True, tile_position=(0, 32 * c),
                )

        # --- element-wise helpers ---
        def prewrite_A(pool, cell, tag):
            """New A bank pre-filled with [gin|r|z] biases (ACT, off-path)."""
            p = pool.tile([128, 512], F32, tag=tag, name=tag)
            nc.scalar.copy(p[:, 0:384], bsb["biasA_" + cell])
            return p

        def new_B(pool, tag):
            return pool.tile([128, 512], F32, tag=tag, name=tag)

        def sig_rz(pA, tag):
            rz = tmp_pool.tile([128, 256], F32, tag=tag + "rz", name="rz")
            nc.scalar.activation(rz, pA[:, 128:384], Sigmoid)
            return rz

        def v_of(rz, pB, tag):
            v = tmp_pool.tile([128, 128], F32, tag=tag + "v", name="v")
            nc.vector.tensor_mul(v, rz[:, 0:128], pB[:, 0:128])
            return v

        def t3_of(v, pA, tag):
            t3 = tmp_pool.tile([128, 128], F32, tag=tag + "t3", name="t3")
            nc.vector.tensor_add(t3, v, pA[:, 0:128])
            return t3

        def tanh_of(t3, tag):
            n = tmp_pool.tile([128, 128], BF16, tag=tag + "n", name="n")
            nc.scalar.activation(n, t3, Tanh)
            return n

        def omz_of(rz, tag):
            omz = tmp_pool.tile([128, 128], BF16, tag=tag + "omz", name="omz")
            nc.gpsimd.tensor_scalar(omz, rz[:, 128:256], -1.0, 1.0, MULT, ADD)
            return omz

        def zh_of(rz, row_prev, tag):
            zh = tmp_pool.tile([128, 128], BF16, tag=tag + "zh", name="zh")
            nc.gpsimd.tensor_mul(zh, rz[:, 128:256], row_prev)
            return zh

        def tail_of(omz, zh, n, tag, eng):
            row = st_pool.tile([128, 128], BF16, tag=tag + "row", name="row")
            if zh is None:
                eng.tensor_mul(row, omz, n)
            else:
                m1 = tmp_pool.tile([128, 128], BF16, tag=tag + "m1", name="m1")
                eng.tensor_mul(m1, omz, n)
                eng.tensor_add(row, m1, zh)
            return row

        def tr_of(row, tag):
            rowT = st_pool.tile([128, 128], BF16, tag=tag + "rowT", name="rowT")
            nc.vector.transpose(rowT, row)
            return rowT

        # --- pipeline state ---
        pgiA, pgiB = {}, {}
        paiA, paiB = {}, {}
        g_row, gT = {}, {}
        a_row, aT = {}, {}
        a_state = {}   # i -> (rz_a, v_a, t3_a, n_a, omz_a, zh_a) transient

        # ---- prologue ----
        pgiA[0] = prewrite_A(ps_giA, "g", "giA")
        mm_x(pgiA[0], lambda k: x_lhsT(0, k), wsb["wgi"])
        pgiB[0] = new_B(ps_giB, "giB")
        mm_biasB(pgiB[0], "g")
        if n_steps > 1:
            pgiA[1] = prewrite_A(ps_giA, "g", "giA")
            mm_x(pgiA[1], lambda k: x_lhsT(1, k), wsb["wgi"])

        # g(0) chain (no hidden side, no zh)
        rz0 = sig_rz(pgiA[0], "g")
        v0 = v_of(rz0, pgiB[0], "g")
        t30 = t3_of(v0, pgiA[0], "g")
        n0 = tanh_of(t30, "g")
        omz0 = omz_of(rz0, "g")
        g_row[0] = tail_of(omz0, None, n0, "g", nc.vector)
        gT[0] = tr_of(g_row[0], "g")
        if n_steps > 2:
            pgiA[2] = prewrite_A(ps_giA, "g", "giA")
        paiA[0] = prewrite_A(ps_aiA, "a", "aiA")

        # ---- steady-state scan ----
        # Per-engine stream order per macro-step i (producing g(i+1), a(i-1)
        # chain tails, and the a(i) matmuls):
        #   PE  : ghid_rz(i+1) | biasB_g+ghid_hn(i+1) | ain(i) | xproj(i+2)
        #         | biasB_a(i)+ahid(i)
        #   ACT : sig_a(i-1) | sig_g(i+1) | tanh_a(i-1) | tanh_g(i+1)
        #         | prewrite_aiA(i+1) | prewrite_giA(i+3)
        #   DVE : v_a,t3_a(i-1) | v_g,t3_g(i+1) | m1_a,row_a,trT_a(i-1)
        #         | m1_g,row_g,trT_g(i+1)
        #   POOL: omz_a,zh_a(i-1) | omz_g,zh_g(i+1)
        def emit_scan():
            for i in range(n_steps + 1):
                # 1. ACT: sig_a(i-1)
                if i >= 1:
                    rz_a = sig_rz(paiA[i - 1], "a")
                # 2. PE: ghid_rz(i+1) -> giA
                if i + 1 < n_steps:
                    mm_h_rz(pgiA[i + 1], gT[i], wsb["wgh"])
                # 3. DVE: v_a, t3_a (i-1)
                if i >= 1:
                    v_a = v_of(rz_a, paiB[i - 1], "a")
                    t3_a = t3_of(v_a, paiA[i - 1], "a")
                # 4. POOL: omz_a, zh_a (i-1)
                if i >= 1:
                    omz_a = omz_of(rz_a, "a")
                    zh_a = zh_of(rz_a, a_row[i - 2], "a") if i >= 2 else None
                # 5. PE: biasB_g + ghid_hn(i+1) -> giB
                if i + 1 < n_steps:
                    pgiB[i + 1] = new_B(ps_giB, "giB")
                    mm_biasB(pgiB[i + 1], "g")
                    mm_h_hn(pgiB[i + 1], gT[i], wsb["wgh"])
                # 6. ACT: sig_g(i+1)
                if i + 1 < n_steps:
                    rz_g = sig_rz(pgiA[i + 1], "g")
                # 7. PE: ain(i) -> aiA
                if i < n_steps:
                    mm_x(paiA[i], lambda k, i=i: gT[i][:, 32 * k:32 * k + 32],
                         wsb["wai"])
                # 8. DVE: v_g, t3_g (i+1)
                if i + 1 < n_steps:
                    v_g = v_of(rz_g, pgiB[i + 1], "g")
                    t3_g = t3_of(v_g, pgiA[i + 1], "g")
                # 9. ACT: tanh_a(i-1)
                if i >= 1:
                    n_a = tanh_of(t3_a, "a")
                # 10. POOL: omz_g, zh_g (i+1)
                if i + 1 < n_steps:
                    omz_g = omz_of(rz_g, "g")
                    zh_g = zh_of(rz_g, g_row[i], "g")
                # 11. PE: xproj(i+2) -> giA (prewritten at step i-1)
                if i + 2 < n_steps:
                    mm_x(pgiA[i + 2], lambda k, s=i + 2: x_lhsT(s, k), wsb["wgi"])
                # 12. DVE: m1_a, row_a, trT_a (i-1)
                if i >= 1:
                    a_row[i - 1] = tail_of(omz_a, zh_a, n_a, "a", nc.vector)
                    a_row.pop(i - 3, None)
                    if i < n_steps:
                        aT[i - 1] = tr_of(a_row[i - 1], "a")
                # 13. ACT: tanh_g(i+1)
                if i + 1 < n_steps:
                    n_g = tanh_of(t3_g, "g")
                # 14. DVE: m1_g, row_g, trT_g (i+1)
                if i + 1 < n_steps:
                    g_row[i + 1] = tail_of(omz_g, zh_g, n_g, "g", nc.vector)
                    gT[i + 1] = tr_of(g_row[i + 1], "g")
                    del pgiA[i + 1]
                    del pgiB[i + 1]
                    g_row.pop(i - 1, None)
                    gT.pop(i, None)
                # 15. PE: biasB_a(i) + ahid(i)
                if i < n_steps:
                    paiB[i] = new_B(ps_aiB, "aiB")
                    mm_biasB(paiB[i], "a")
                    if i >= 1:
                        mm_h_rz(paiA[i], aT[i - 1], wsb["wah"])
                        mm_h_hn(paiB[i], aT[i - 1], wsb["wah"])
                        aT.pop(i - 1, None)
                # 16. ACT: prewrite aiA(i+1)
                if i + 1 < n_steps:
                    paiA[i + 1] = prewrite_A(ps_aiA, "a", "aiA")
                # 17. ACT: prewrite giA(i+3) (for xproj(i+3) emitted next step)
                if i + 3 < n_steps:
                    pgiA[i + 3] = prewrite_A(ps_giA, "g", "giA")
                # cleanup consumed psum refs for the a cell
                if i >= 1:
                    paiA.pop(i - 1, None)
                    paiB.pop(i - 1, None)

        loop_cm = tc.For_i(0, repeat, 1) if repeat > 1 else nullcontext()
        with loop_cm:
            emit_scan()

        # epilogue: final AUGRU state -> fp32 -> DRAM
        final = a_row[n_steps - 1]
        out_row = tmp_pool.tile([128, 128], F32, tag="outrow")
        nc.scalar.copy(out_row, final)
        out_ap = bass_mod.AP(
            tensor=out[:].tensor,
            offset=0,
            ap=[[128, 4], [H, BL], [1, 128]],
        )
        nc.sync.dma_start(out=out_ap, in_=out_row)

    nc.compile()
    return nc


def _get_program(n_steps=T):
    key = ("prog", n_steps)
    if key not in _CACHE:
        _CACHE[key] = _build_program(n_steps)
    return _CACHE[key]


# ---------------------------------------------------------------------------
# Entry point
# ---------------------------------------------------------------------------

def _make_consts(inputs):
    augru_Wih = np.asarray(inputs["augru_Wih"], np.float64)
    A1 = augru_Wih[:, :H]
    A2 = augru_Wih[:, H:]
    w_fused = A1 + A2 @ np.asarray(inputs["v_W"], np.float64)
    b_ai = np.asarray(inputs["augru_bih"], np.float64) + A2 @ np.asarray(
        inputs["v_b"], np.float64
    )
    b_ah = np.asarray(inputs["augru_bhh"], np.float64)
    gru_bih = np.asarray(inputs["gru_bih"], np.float64)
    gru_bhh = np.asarray(inputs["gru_bhh"], np.float64)

    def f32(x):
        return np.ascontiguousarray(x, dtype=np.float32)

    def bf(x):
        return np.ascontiguousarray(x.astype(np.float32)).astype(BF16NP)

    small = np.zeros((1, 1056), np.float64)
    small[0, 0:32] = 1.0
    small[0, 32:544] = _bias_B_row(gru_bhh)[0]
    small[0, 544:1056] = _bias_B_row(b_ah)[0]

    return {
        "wgi": bf(_arrange_w(np.asarray(inputs["gru_Wih"], np.float64), True)),
        "wgh": bf(_arrange_w(np.asarray(inputs["gru_Whh"], np.float64), False)),
        "wai": bf(_arrange_w(w_fused, True)),
        "wah": bf(_arrange_w(np.asarray(inputs["augru_Whh"], np.float64), False)),
        "biasA_g": f32(_bias_A(gru_bih, gru_bhh)),
        "biasA_a": f32(_bias_A(b_ai, b_ah)),
        "small": bf(small),
    }


def _make_in_maps(inputs):
    seq_emb = np.asarray(inputs["seq_emb"], np.float32)
    consts = _make_consts(inputs)
    return [
        {"xt": _arrange_x(seq_emb[c * BL:(c + 1) * BL]), **consts}
        for c in range(N_CORES)
    ]


def _prep_and_run(trace=False, **inputs):
    from concourse.bass_utils import run_bass_kernel_spmd

    in_maps = _make_in_maps(inputs)
    nc = _get_program()
    res = run_bass_kernel_spmd(nc, in_maps, list(range(N_CORES)), trace=trace)
    out = np.concatenate([res.results[c]["out"] for c in range(N_CORES)], axis=0)
    return out.astype(np.float32), res


def kernel(**inputs):
    out, _ = _prep_and_run(**inputs)
    return out


def kernel_traced(**inputs):
    """Like kernel() but profiles the run; returns (output, BassKernelResults)."""
    return _prep_and_run(**inputs, trace=True)
